# revision 35
# baseline (speedup 1.0000x reference)
"""Causal multi-head attention (B=2, H=16, S=2048, D=128, fp32) on 8 TRN2
NeuronCores.

Sharding: batch*heads = 32 (b,h) pairs, 4 per core (pure data/head parallel,
no collectives). Software-pipelined for the in-order per-engine queues:

  - Q,K are PE-transposed into [d, s] layout; the transpose batch for
    query-superblock s+1 borrows an st PSUM slot and is emitted between
    score groups of superblock s, so it never stalls the score/exp
    ping-pong. Only chunk 0 of Q/K is DMA'd before the first transposes.
  - Scores are computed *transposed* (st[k, q] = K_blk @ Q^T) with float32r
    matmuls into triple-buffered 2-bank [128, 1024] PSUM supertiles (2 key
    tiles each). One ScalarE exp per supertile. The two diagonal pairs use
    narrowed matmuls (N=512+384 and N=256+256) packed tightly so the
    causally-dead query prefixes are neither computed nor exp'd.
  - Causal mask: only the diagonal 128x128 sub-block of each diagonal score
    tile gets a NEG mask add; sub-blocks strictly above the diagonal are
    never read (the PV loop skips kb > t).
  - PV runs in natural output layout: out[q, d] += pt_sub[k, q].T @ v[k, d]
    with pt stationary (bf16, FWL) and V natural moving with a ones-column
    appended (N=129): column 128 of each PSUM accumulator collects the
    softmax row-sum for free. PV chains for superblock s-1 are split into
    <=5-matmul pieces and interleaved (capped at 3 per score group) between
    the score groups of s, so the PE stays dense (HAM at K=8/8) without
    head-of-line-blocking the score matmuls that feed ScalarE.
  - Row-sum reciprocals via DVE over [128, 2] PSUM column slices; normalize
    via tensor_scalar_mul with a [128, 1] scalar AP; fp32 natural-layout
    output DMA'd straight out. No output transposes, no broadcast matmuls,
    no vector tree-adds, no 1-partition reciprocals.
  - PSUM budget: st 3x[128,1024] (6 banks) + oa [128,2,512] (2 banks) = 8.
    All constants ride one [128, 258] DMA; a dummy exp preloads the ACT
    table set off the critical path.
"""

import numpy as np
import ml_dtypes
from contextlib import ExitStack

B, H, S, D = 2, 16, 2048, 128
NCORES = 8
HPC = (B * H) // NCORES  # heads per core
P = 128                  # tile partition size
NQS = 512                # query superblock width
NT = S // P              # 16 key tiles per head
NS = S // NQS            # 4 query superblocks per head
KPS = NQS // P           # 4 key tiles per query superblock
NQW = 256                # score matmul query window (f32r min fast width)
VAUG = 130               # vb_aug row stride (129 used, padded for alignment)
PVCH = 5                 # max matmuls per interleaved PV piece
NEG = -1.0e9

_cache = {}


def _build():
    import concourse.tile as tile
    from concourse import bacc, mybir

    f32 = mybir.dt.float32
    f32r = mybir.dt.float32r
    bf16 = mybir.dt.bfloat16
    Exp = mybir.ActivationFunctionType.Exp

    nc = bacc.Bacc("TRN2", target_bir_lowering=False, debug=False,
                   num_devices=NCORES)
    q_ext = nc.dram_tensor("query", [HPC, S, D], f32, kind="ExternalInput").ap()
    k_ext = nc.dram_tensor("key", [HPC, S, D], f32, kind="ExternalInput").ap()
    v_ext = nc.dram_tensor("value", [HPC, S, D], f32, kind="ExternalInput").ap()
    # all constants in one DMA: [sb | ng | diagm | ident] = [128, 258]
    cc_ext = nc.dram_tensor("cconst", [P, 2 + 2 * P], f32,
                            kind="ExternalInput").ap()
    out_ext = nc.dram_tensor("out", [HPC, S, D], f32, kind="ExternalOutput").ap()

    with tile.TileContext(nc) as tc, ExitStack() as ctx:
        consts = ctx.enter_context(tc.tile_pool(name="consts", bufs=1))
        cc_t = consts.tile([P, 2 + 2 * P], f32, tag="cc")
        nc.sync.dma_start(cc_t[:], cc_ext[:])
        sb_t = cc_t[:, 0:1]
        ng_t = cc_t[:, 1:2]
        dm_t = cc_t[:, 2:2 + P]
        id_t = cc_t[:, 2 + P:2 + 2 * P]
        # dummy exp to pull ACT_TABLE_LOAD off the critical path
        warm = consts.tile([P, 1], f32, tag="warm")
        nc.scalar.activation(warm[:], ng_t, Exp)

        p_nat = ctx.enter_context(tc.tile_pool(name="nat", bufs=2))
        p_tt = ctx.enter_context(tc.tile_pool(name="tt", bufs=2))
        p_pt = ctx.enter_context(tc.tile_pool(name="pt", bufs=12))
        p_osb = ctx.enter_context(tc.tile_pool(name="osb", bufs=2))
        p_rs = ctx.enter_context(tc.tile_pool(name="rs", bufs=2))
        # PSUM: st 2x[128,1024](4 banks) + tp 2x[128,512](2) + oa 1x2banks = 8
        p_ps = ctx.enter_context(tc.tile_pool(name="ps", bufs=1, space="PSUM"))

        heads = {}

        def dma_head_first(h):
            """Only chunk 0 of Q/K: the minimum for the first transposes."""
            qn = p_nat.tile([P, NT, P], f32, tag="qn", name=f"qn{h}")
            kn = p_nat.tile([P, NT, P], f32, tag="kn", name=f"kn{h}")
            for nat, ext in ((qn, q_ext), (kn, k_ext)):
                nc.sync.dma_start(
                    nat[:, 0:4, :],
                    ext[h, 0:NQS, :].rearrange("(t p) d -> p t d", p=P))
            qt = p_tt.tile([P, S], f32r, tag="qt", name=f"qt{h}")
            kt = p_tt.tile([P, S], f32r, tag="kt", name=f"kt{h}")
            heads[h] = dict(qn=qn, kn=kn, qt=qt, kt=kt, kbmap={}, oa=None)

        def dma_head_rest(h):
            hd = heads[h]
            for c in range(1, 4):
                for nat, ext in ((hd["qn"], q_ext), (hd["kn"], k_ext)):
                    nc.sync.dma_start(
                        nat[:, 4 * c:4 * c + 4, :],
                        ext[h, c * NQS:(c + 1) * NQS, :].rearrange(
                            "(t p) d -> p t d", p=P))
            vn = p_nat.tile([P, NT, P], f32, tag="vn", name=f"vn{h}")
            nc.sync.dma_start(vn[:], v_ext[h].rearrange("(t p) d -> p t d", p=P))
            vb = p_tt.tile([P, NT, VAUG], bf16, tag="vb", name=f"vb{h}")
            nc.gpsimd.memset(vb[:, :, P:P + 1], 1.0)
            for c in range(4):
                nc.gpsimd.tensor_copy(vb[:, 4 * c:4 * c + 4, 0:P],
                                      vn[:, 4 * c:4 * c + 4, :])
            hd["vn"] = vn
            hd["vb"] = vb

        def dma_head(h):
            dma_head_first(h)
            dma_head_rest(h)

        def transpose_batch(h, s):
            """PE-transpose Q and K chunk s into qt/kt columns, borrowing one
            st slot (Q batch in cols 0-511, K batch in 512-1023)."""
            hd = heads[h]
            tp = p_ps.tile([P, 2 * NQS], f32, tag="st", bufs=3,
                           name=f"tp{h}_{s}")
            for w, (nat, tr) in enumerate(((hd["qn"], hd["qt"]),
                                           (hd["kn"], hd["kt"]))):
                for jj in range(4):
                    t = 4 * s + jj
                    nc.tensor.transpose(
                        tp[:, w * NQS + jj * P:w * NQS + (jj + 1) * P],
                        nat[:, t, :], id_t)
                nc.vector.tensor_copy(tr[:, s * NQS:(s + 1) * NQS],
                                      tp[:, w * NQS:(w + 1) * NQS])

        def score_group(h, s, group):
            """Score MMs + diag mask + exp for one supertile of superblock s.
            group = (specs, exp_end); spec = (kb, dst_off, width, qt_off,
            mask_c0 or None, pv_base)."""
            hd = heads[h]
            specs, end = group
            st = p_ps.tile([P, 2 * NQS], f32, tag="st", bufs=3,
                           name=f"st{h}_{s}_{specs[0][0]}")
            for kb, doff, w, qoff, mc0, base in specs:
                nc.tensor.matmul(
                    st[:, doff:doff + w],
                    hd["kt"][:, kb * P:(kb + 1) * P],
                    hd["qt"][:, s * NQS + qoff:s * NQS + qoff + w],
                    start=True, stop=True,
                )
                if mc0 is not None:
                    nc.vector.tensor_add(
                        st[:, mc0:mc0 + P], st[:, mc0:mc0 + P], dm_t)
            pt = p_pt.tile([P, 2 * NQS], bf16, tag="pt", bufs=16,
                           name=f"pt{h}_{s}_{specs[0][0]}")
            nc.scalar.activation(pt[:, 0:end], st[:, 0:end], Exp,
                                 bias=ng_t, scale=sb_t)
            for kb, doff, w, qoff, mc0, base in specs:
                hd["kbmap"][(s, kb)] = (pt, base)

        def pv_pieces(h, s):
            """Closures: oa alloc, <=PVCH-matmul accumulation pieces per
            q-block of superblock s, then finalize (recip+mul+dma)."""
            hd = heads[h]

            def start(hd=hd, h=h, s=s):
                hd["oa"] = p_ps.tile([P, 2, NQS], f32, tag="oa",
                                     name=f"oa{h}_{s}")

            def piece(u, lo, hi, hd=hd, s=s):
                t = KPS * s + u
                oa = hd["oa"]
                dst = oa[:, u // 2,
                         (u % 2) * (P + 1):(u % 2) * (P + 1) + P + 1]
                for kb in range(lo, hi):
                    pt, base = hd["kbmap"][(s, kb)]
                    nc.tensor.matmul(
                        dst,
                        pt[:, base + u * P:base + u * P + P],
                        hd["vb"][:, kb, 0:P + 1],
                        start=(kb == 0), stop=(kb == t),
                    )

            def finalize(hd=hd, h=h, s=s):
                oa = hd["oa"]
                rs = p_rs.tile([P, 2, 2], f32, tag="rs", name=f"rs{h}_{s}")
                for m in range(2):
                    nc.vector.reciprocal(
                        rs[:, :, m],
                        oa[:, :, m * (P + 1) + P:m * (P + 1) + P + 1])
                osb = p_osb.tile([P, KPS, P], f32, tag="osb",
                                 name=f"osb{h}_{s}")
                for u in range(KPS):
                    nc.vector.tensor_scalar_mul(
                        osb[:, u, :],
                        oa[:, u // 2, (u % 2) * (P + 1):(u % 2) * (P + 1) + P],
                        rs[:, u // 2, (u % 2):(u % 2) + 1],
                    )
                nc.sync.dma_start(
                    out_ext[h, s * NQS:(s + 1) * NQS, :].rearrange(
                        "(j p) d -> p j d", p=P),
                    osb[:],
                )

            pieces = [start]
            for u in range(KPS):
                t = KPS * s + u
                for lo in range(0, t + 1, PVCH):
                    pieces.append(lambda u=u, lo=lo, hi=min(lo + PVCH, t + 1):
                                  piece(u, lo, hi))
            pieces.append(finalize)
            return pieces

        def kb_pairs(s):
            """Score supertile groups for superblock s. Off-diagonal pairs
            are full N=512 tiles; the two diagonal pairs use narrowed matmuls
            (causally-dead query prefixes skipped) packed tightly so the exp
            span shrinks: (4s,4s+1) -> 896 cols, (4s+2,4s+3) -> 512 cols."""
            d = KPS * s
            gs = [([(kb, 0, NQS, 0, None, 0),
                    (kb + 1, NQS, NQS, 0, None, NQS)], 2 * NQS)
                  for kb in range(0, 4 * s, 2)]
            gs.append(([(d, 0, NQS, 0, 0, 0),
                        (d + 1, NQS, 3 * P, P, NQS, 3 * P)], 7 * P))
            gs.append(([(d + 2, 0, NQW, NQW, 0, -NQW),
                        (d + 3, NQW, NQW, NQW, 3 * P, 0)], NQS))
            return gs

        # flat software-pipelined schedule over (h, s)
        pv_fifo = []
        dma_head_first(0)
        transpose_batch(0, 0)
        dma_head_rest(0)
        for h in range(HPC):
            if h + 1 < HPC:
                dma_head(h + 1)
            for s in range(NS):
                pairs = kb_pairs(s)
                # transposes for the next superblock (or next head's s=0)
                tposes = ([(h, s + 1)] if s + 1 < NS
                          else ([(h + 1, 0)] if h + 1 < HPC else []))
                for g, pair in enumerate(pairs):
                    score_group(h, s, pair)
                    # cap pops so queued PV never delays the exp feed; the
                    # remainder drains after the last score group
                    npop = min(3, ((len(pv_fifo) + len(pairs) - g - 1)
                                   // (len(pairs) - g)))
                    for _ in range(npop):
                        pv_fifo.pop(0)()
                    if g == 0 and tposes:
                        transpose_batch(*tposes[0])
                while pv_fifo:
                    pv_fifo.pop(0)()
                pv_fifo = pv_pieces(h, s)
        while pv_fifo:
            pv_fifo.pop(0)()
    nc.compile()
    return nc


def get_nc():
    if "nc" not in _cache:
        _cache["nc"] = _build()
    return _cache["nc"]


def make_in_maps(query, key, value, scale):
    q = np.ascontiguousarray(np.asarray(query, dtype=np.float32)).reshape(B * H, S, D)
    k = np.ascontiguousarray(np.asarray(key, dtype=np.float32)).reshape(B * H, S, D)
    v = np.ascontiguousarray(np.asarray(value, dtype=np.float32)).reshape(B * H, S, D)
    sc = float(np.asarray(scale).reshape(-1)[0])

    # combined consts [sb | ng | diagm | ident] = [128, 258]
    # diagm[dk, dq] = 0 if dq >= dk else NEG (causal within diagonal block)
    dks = np.arange(P)[:, None]
    dqs = np.arange(P)[None, :]
    diagm = np.where(dqs >= dks, 0.0, NEG).astype(np.float32)
    cconst = np.concatenate([
        np.full((P, 1), sc, dtype=np.float32),
        np.full((P, 1), -50.0, dtype=np.float32),
        diagm,
        np.eye(P, dtype=np.float32),
    ], axis=1)

    in_maps = []
    for c in range(NCORES):
        sl = slice(c * HPC, (c + 1) * HPC)
        in_maps.append({
            "query": np.ascontiguousarray(q[sl]),
            "key": np.ascontiguousarray(k[sl]),
            "value": np.ascontiguousarray(v[sl]),
            "cconst": cconst,
        })
    return in_maps


def kernel(query, key, value, scale):
    from concourse.bass_utils import run_bass_kernel_spmd

    nc = get_nc()
    in_maps = make_in_maps(query, key, value, scale)
    res = run_bass_kernel_spmd(nc, in_maps, core_ids=list(range(NCORES)))
    out = np.empty((B * H, S, D), dtype=np.float32)
    for c in range(NCORES):
        out[c * HPC:(c + 1) * HPC] = res.results[c]["out"]
    return out.reshape(B, H, S, D)
